# revision 20
# baseline (speedup 1.0000x reference)
"""CapsuleLayer (dynamic routing) Trainium2 Bass kernel.

Full inputs:  x [128, 512, 256] f32, W [32, 512, 16, 256] f32
Full output:  [128, 32, 16] f32

Sharding: split the input-capsule dim N=512 across 8 cores (64 each).
Each core computes its slice of inputs_hat = einsum('bni,mndi->bmnd')
in bf16, keeps it SBUF-resident as [b=128 part, (n_loc, d, m) free],
runs the 3 routing iterations locally (softmax over m is fully local),
and the per-core partial s = sum_n c*inputs_hat is AllReduced (256KB)
once per iteration.  W and x are each read from HBM exactly once in
aggregate (~21MB per core in bf16), the memory roofline.

v2: inputs_hat stored bf16 (2x DVE tensor_tensor mode); the segmented
reductions are balanced binary tree-adds (tensor_tensor at 2x) instead
of 1x-capped tensor_reduce; softmax exp runs on the scalar engine; DMA
layouts are host-repacked so every HBM transfer has >=2KB contiguous
lines and >=0.5MB size.
"""

import sys

sys.path.insert(0, "/opt/trn_rl_repo")

import numpy as np

import concourse.bacc as bacc
import concourse.mybir as mybir
import concourse.tile as tile
from concourse.bass_utils import run_bass_kernel_spmd

N_CORES = 8
B, N, I = 128, 512, 256
M, D = 32, 16
DM = D * M                 # ih free layout is (d, m): m innermost
NL = N // N_CORES          # 64 local input capsules per core
EPS = 1e-7
F32 = mybir.dt.float32
BF16 = mybir.dt.bfloat16

NB = 8                     # n-block size per wt DMA (may be overridden by cfg)
CS = 16                    # n-chunk size for routing passes
NCH = NL // CS
MH = 2                     # m-halves (v3): AR + squash + b-update pipelined per half
MW = M // MH               # 16 m per half
DMH = D * MW               # 256 free elems per half
NB3 = 16                   # v3 wt DMA n-block (2MB per transfer)
CSR = 64                   # v3 routing n-chunk (one chunk per m-half)

# debug/profiling knobs (defaults = full kernel)
_cfg = {
    "routing": True, "iters": (2, 3), "reps": 1,
    "mul_eng": "dve",      # leaf-multiply engine: dve | gps | split
    "tree_eng": "dve",     # tree-add engine: dve | split
    "s1_dup": True,        # accumulate iter-1 s via duplicated PE matmuls
    "copy_eng": "dve",     # inputs_hat PSUM->SBUF copy engine: dve | act
    "nb": NB,              # n-block size per wt DMA (8 -> 2MB, 16 -> 4MB)
    "ver": 3,              # 2 = monolithic-m body, 3 = m-half pipelined body
}

import json as _json
import os as _os

_cfg.update(_json.loads(_os.environ.get("KCFG", "{}")))
if "iters" in _cfg:
    _cfg["iters"] = tuple(_cfg["iters"])


def _squash(tc, pool, s_src, scale_pre, eps_t):
    """o = squash(s) over d; s layout [128, (d, m)]. Returns o tile (f32)."""
    nc = tc.nc
    ssb = pool.tile([128, DM], F32, tag="ssb")
    nc.scalar.mul(out=ssb, in_=s_src, mul=scale_pre)  # copy (+scale) to SBUF
    sq = pool.tile([128, DM], F32, tag="sq")
    nc.vector.tensor_mul(sq, ssb, ssb)
    s2 = pool.tile([128, M], F32, tag="s2")
    nc.vector.tensor_reduce(
        s2, sq.rearrange("p (d m) -> p m d", d=D),
        axis=mybir.AxisListType.X, op=mybir.AluOpType.add,
    )
    rt = pool.tile([128, M], F32, tag="rt")
    nc.scalar.activation(rt, s2, mybir.ActivationFunctionType.Sqrt,
                         bias=eps_t[:, 0:1])
    one_p = pool.tile([128, M], F32, tag="one_p")
    nc.vector.tensor_scalar_add(one_p, s2, 1.0)
    den = pool.tile([128, M], F32, tag="den")
    nc.vector.tensor_mul(den, one_p, rt)
    rec = pool.tile([128, M], F32, tag="rec")
    nc.vector.reciprocal(rec, den)
    scl = pool.tile([128, M], F32, tag="scl")
    nc.vector.tensor_mul(scl, s2, rec)      # scale = s2/(1+s2)/sqrt(s2+eps)
    o = pool.tile([128, DM], F32, tag="o")
    nc.vector.tensor_mul(
        o.rearrange("p (d m) -> p d m", d=D),
        ssb.rearrange("p (d m) -> p d m", d=D),
        scl.unsqueeze(1).broadcast_to([128, D, M]),
    )
    return o


def _allreduce(tc, dram_pool, sb_pool, src, idx, n_cores=N_CORES):
    """AllReduce [128, DM] f32 across the cores. Returns SBUF tile."""
    nc = tc.nc
    bin_ = dram_pool.tile([128, DM], F32, tag=f"arin{idx}")
    bout = dram_pool.tile([128, DM], F32, tag=f"arout{idx}")
    nc.sync.dma_start(out=bin_[:], in_=src)
    if n_cores > 1 and not _cfg.get("no_cc"):
        nc.gpsimd.collective_compute(
            "AllReduce", mybir.AluOpType.add,
            replica_groups=[list(range(n_cores))],
            ins=[bin_.opt()], outs=[bout.opt()],
        )
    else:
        nc.sync.dma_start(out=bout[:], in_=bin_[:])  # sim stand-in
    dst = sb_pool.tile([128, DM], F32, tag="sglob")
    nc.sync.dma_start(out=dst[:], in_=bout[:])
    return dst


def _mul_eng(nc, k):
    return {"dve": nc.vector, "gps": nc.gpsimd,
            "split": nc.gpsimd if k % 4 == 3 else nc.vector}[_cfg["mul_eng"]]


def _tree_eng(nc, k):
    return {"dve": nc.vector,
            "split": nc.gpsimd if k % 4 == 3 else nc.vector}[_cfg["tree_eng"]]


def _bench_body(tc, out_ap):
    """Microbenchmark: N ops of one pattern per rep (cfg bench=pattern,N)."""
    nc = tc.nc
    pat, nops = _cfg["bench"].split(",")
    nops = int(nops)
    with tc.tile_pool(name="bp", bufs=1) as bp:
        FD = 16384
        a = bp.tile([128, FD], BF16, tag="a")
        b = bp.tile([128, FD], BF16, tag="b")
        o = bp.tile([128, FD], BF16, tag="o")
        af = bp.tile([128, FD // 2], F32, tag="af")
        bf = bp.tile([128, FD // 2], F32, tag="bf")
        of = bp.tile([128, FD // 2], F32, tag="of")
        sm = bp.tile([128, DMH], BF16, tag="sm")
        red = bp.tile([128, 64], F32, tag="red")
        nc.vector.memset(a, 0.5)
        nc.vector.memset(b, 0.25)
        nc.vector.memset(af, 0.5)
        nc.vector.memset(bf, 0.25)
        nc.vector.memset(sm, 0.5)
        for i in range(nops):
            if pat == "tt_dense":
                nc.vector.tensor_mul(o, a, b)
            elif pat == "tt_dense_f32":
                nc.vector.tensor_mul(of, af, bf)
            elif pat == "tt_bcast":
                # [p, 64, 256] with in1 broadcast over n (the bup leaf shape)
                nc.vector.tensor_mul(
                    o.rearrange("p (n f) -> p n f", n=64),
                    a.rearrange("p (n f) -> p n f", n=64),
                    sm.unsqueeze(1).broadcast_to([128, 64, DMH]))
            elif pat == "tt_bcast4":
                # 4-dim strided mul (the s-step leaf shape):
                # in1 = c[p, n, m] broadcast over d
                cm = b.rearrange("p (n m x) -> p n m x", n=64, m=16)[:, :, :, 0]
                nc.vector.tensor_mul(
                    o.rearrange("p (n d m) -> p n d m", n=64, d=D),
                    a.rearrange("p (n d m) -> p n d m", n=64, d=D),
                    cm.unsqueeze(2).broadcast_to([128, 64, D, 16]))
            elif pat == "tt_tree":
                # 4-dim strided tree add (the d-tree shape)
                av = a.rearrange("p (n d m) -> p n d m", n=64, d=D)
                nc.vector.tensor_add(
                    o.rearrange("p (n d m) -> p n d m", n=128, d=8)[:, 0:64],
                    av[:, :, 0:8, :], av[:, :, 8:16, :])
            elif pat == "reduce":
                nc.vector.tensor_reduce(
                    red, a.rearrange("p (n f) -> p n f", n=64),
                    axis=mybir.AxisListType.X, op=mybir.AluOpType.add)
            elif pat == "gps_tt":
                nc.gpsimd.tensor_mul(o, a, b)
            elif pat == "act_copy":
                nc.scalar.copy(o, a[:])
        nc.sync.dma_start(out=out_ap, in_=af[:, 0:DM])


def _body(tc, out_ap, wtr, xtr, n_cores=N_CORES):
    if _cfg.get("bench"):
        for _rep in range(_cfg.get("reps", 1)):
            _bench_body(tc, out_ap)
        return
    body = _body_once_v3 if _cfg["ver"] == 3 else _body_once
    for _rep in range(_cfg.get("reps", 1)):
        body(tc, out_ap, wtr, xtr, n_cores)


def _body_once(tc, out_ap, wtr, xtr, n_cores=N_CORES):
    nc = tc.nc
    X = mybir.AxisListType.X
    ADD = mybir.AluOpType.add

    with tc.tile_pool(name="persist", bufs=1) as persist, \
         tc.tile_pool(name="dram", bufs=1, space="DRAM") as dram:
        ihb = persist.tile([128, NL, DM], BF16)    # inputs_hat bf16, 64KB/part
        eps_t = persist.tile([128, 1], F32, tag="eps")
        nc.vector.memset(eps_t, EPS)

        # ---------------- einsum phase ----------------
        with tc.tile_pool(name="psum_s1", bufs=1, space="PSUM") as psum_s1:
            if _cfg["s1_dup"]:
                s1_ps = psum_s1.tile([128, DM], F32)  # sum_n ih (PE-accum)
            with tc.tile_pool(name="xt_pool", bufs=1) as xt_pool, \
                 tc.tile_pool(name="wt_pool", bufs=2) as wt_pool, \
                 tc.tile_pool(name="psum_mm", bufs=4, space="PSUM") as psum_mm:
                nbs = _cfg["nb"]
                xt_all = xt_pool.tile([128, 2, NL, B], BF16)   # 32KB/part
                xt_eng = nc.scalar if _cfg["copy_eng"] == "act" else nc.sync
                xt_eng.dma_start(
                    out=xt_all[:],
                    in_=xtr.rearrange("(h p) n b -> p h n b", p=128),
                )
                for nb in range(NL // nbs):
                    wt_t = wt_pool.tile([128, 2, nbs, DM], BF16,
                                        tag=f"wt_{nb % 2}")
                    if _cfg["copy_eng"] == "act":
                        # ACT queue is kept free for the ihb copies; the wt
                        # stream is serial on the sync ring (it is the bulk
                        # HBM stream and back-to-back regardless)
                        dma_eng = nc.sync
                    else:
                        dma_eng = nc.sync if nb % 2 == 0 else nc.scalar
                    dma_eng.dma_start(
                        out=wt_t[:],
                        in_=wtr[nb].rearrange("(h p) n m -> p h n m", p=128))
                    for j in range(nbs):
                        n = nb * nbs + j
                        ps = psum_mm.tile([128, DM], F32)
                        nc.tensor.matmul(ps, lhsT=xt_all[:, 0, n, :],
                                         rhs=wt_t[:, 0, j, :],
                                         start=True, stop=False)
                        if _cfg["s1_dup"]:
                            nc.tensor.matmul(s1_ps, lhsT=xt_all[:, 0, n, :],
                                             rhs=wt_t[:, 0, j, :],
                                             start=(n == 0), stop=False,
                                             skip_group_check=True)
                        nc.tensor.matmul(ps, lhsT=xt_all[:, 1, n, :],
                                         rhs=wt_t[:, 1, j, :],
                                         start=False, stop=True)
                        if _cfg["s1_dup"]:
                            nc.tensor.matmul(s1_ps, lhsT=xt_all[:, 1, n, :],
                                             rhs=wt_t[:, 1, j, :],
                                             start=False, stop=(n == NL - 1),
                                             skip_group_check=True)
                        if _cfg["copy_eng"] == "act":
                            nc.scalar.copy(ihb[:, n, :], ps[:])
                        else:
                            nc.vector.tensor_copy(ihb[:, n, :], ps)

            # -------- iteration 1 (uniform c): s1 = sum_n ih / M --------
            with tc.tile_pool(name="rs0", bufs=1) as rs0:
                s1_sb = rs0.tile([128, DM], F32, tag="s1_sb")
                if _cfg["s1_dup"]:
                    nc.scalar.mul(out=s1_sb, in_=s1_ps[:], mul=1.0 / M)
                else:
                    # binary tree over n on DVE (bf16 2x mode)
                    w1 = rs0.tile([128, 32, DM], BF16, tag="w1")
                    nc.vector.tensor_add(w1, ihb[:, 0:32, :], ihb[:, 32:64, :])
                    w2 = rs0.tile([128, 16, DM], BF16, tag="w2")
                    nc.vector.tensor_add(w2, w1[:, 0:16, :], w1[:, 16:32, :])
                    w3 = rs0.tile([128, 8, DM], BF16, tag="w3")
                    nc.vector.tensor_add(w3, w2[:, 0:8, :], w2[:, 8:16, :])
                    w4 = rs0.tile([128, 4, DM], BF16, tag="w4")
                    nc.vector.tensor_add(w4, w3[:, 0:4, :], w3[:, 4:8, :])
                    w5 = rs0.tile([128, 2, DM], BF16, tag="w5")
                    nc.vector.tensor_add(w5, w4[:, 0:2, :], w4[:, 2:4, :])
                    s1f = rs0.tile([128, DM], F32, tag="s1f")
                    nc.vector.tensor_add(s1f, w5[:, 0, :], w5[:, 1, :])
                    nc.scalar.mul(out=s1_sb, in_=s1f[:], mul=1.0 / M)
                s1g = _allreduce(tc, dram, rs0, s1_sb[:], 0, n_cores)
                o = _squash(tc, persist, s1g[:], 1.0, eps_t)
                o_bf = persist.tile([128, DM], BF16, tag="o_bf")
                nc.scalar.copy(o_bf, o[:])

        if not _cfg["routing"]:
            nc.sync.dma_start(out=out_ap, in_=o[:])
            return

        # ---------------- routing iterations 2..3 ----------------
        with tc.tile_pool(name="rp", bufs=1) as rp, \
             tc.tile_pool(name="rsmall", bufs=2) as rsmall, \
             tc.tile_pool(name="tmp", bufs=2) as tmp:
            b_log = rp.tile([128, NL, M], F32)     # routing logits
            for it in _cfg["iters"]:
                first_it = it == _cfg["iters"][0]
                # ---- b-update: b_log (+)= sum_d o * ih (binary tree) ----
                for k in range(NCH):
                    ksl = slice(k * CS, (k + 1) * CS)
                    t = tmp.tile([128, CS, DM], BF16, tag="t")
                    _mul_eng(nc, k).tensor_mul(
                        t, ihb[:, ksl, :],
                        o_bf.unsqueeze(1).broadcast_to([128, CS, DM]),
                    )
                    tv = t.rearrange("p n (d m) -> p n d m", d=D)
                    s1t = tmp.tile([128, 8 * CS * M], BF16, tag="s1")
                    l1 = s1t.rearrange("p (n d m) -> p n d m", n=CS, d=8)
                    _tree_eng(nc, k).tensor_add(
                        l1, tv[:, :, 0:8, :], tv[:, :, 8:16, :])
                    s2t = tmp.tile([128, 4 * CS * M], BF16, tag="s2")
                    l2 = s2t.rearrange("p (n d m) -> p n d m", n=CS, d=4)
                    _tree_eng(nc, k).tensor_add(
                        l2, l1[:, :, 0:4, :], l1[:, :, 4:8, :])
                    s3t = tmp.tile([128, 2 * CS * M], BF16, tag="s3")
                    l3 = s3t.rearrange("p (n d m) -> p n d m", n=CS, d=2)
                    _tree_eng(nc, k).tensor_add(
                        l3, l2[:, :, 0:2, :], l2[:, :, 2:4, :])
                    if first_it:
                        nc.vector.tensor_add(b_log[:, ksl, :],
                                             l3[:, :, 0, :], l3[:, :, 1, :])
                    else:
                        bup = tmp.tile([128, CS, M], F32, tag="bup")
                        nc.vector.tensor_add(bup, l3[:, :, 0, :],
                                             l3[:, :, 1, :])
                        nc.vector.tensor_add(b_log[:, ksl, :],
                                             b_log[:, ksl, :], bup)
                # ---- softmax over m (innermost free dim) ----
                e_t = rp.tile([128, NL, M], BF16, tag="e_t")
                nc.scalar.activation(e_t, b_log,
                                     mybir.ActivationFunctionType.Exp)
                zt = rsmall.tile([128, NL], F32, tag="zt")
                nc.vector.tensor_reduce(zt, e_t, axis=X, op=ADD)
                rz = rsmall.tile([128, NL], F32, tag="rz")
                nc.vector.reciprocal(rz, zt)
                rzb = rsmall.tile([128, NL], BF16, tag="rzb")
                nc.scalar.copy(rzb, rz[:])
                c_t = e_t    # normalize in place: c = e * (1/Z)
                nc.vector.tensor_mul(
                    c_t, e_t, rzb.unsqueeze(2).broadcast_to([128, NL, M]))
                # ---- s-step: s = sum_n c * ih (binary tree over n) ----
                s_parts = rp.tile([128, NCH, DM], F32, tag="s_parts")
                for k in range(NCH):
                    ksl = slice(k * CS, (k + 1) * CS)
                    t2 = tmp.tile([128, CS, DM], BF16, tag="t")
                    _mul_eng(nc, k).tensor_mul(
                        t2.rearrange("p n (d m) -> p n d m", d=D),
                        ihb[:, ksl, :].rearrange("p n (d m) -> p n d m", d=D),
                        c_t[:, ksl, :].unsqueeze(2).broadcast_to(
                            [128, CS, D, M]),
                    )
                    s1t = tmp.tile([128, 8 * CS * M], BF16, tag="s1")
                    v1 = s1t.rearrange("p (n f) -> p n f", n=8)
                    _tree_eng(nc, k).tensor_add(
                        v1, t2[:, 0:8, :], t2[:, 8:16, :])
                    s2t = tmp.tile([128, 4 * CS * M], BF16, tag="s2")
                    v2 = s2t.rearrange("p (n f) -> p n f", n=4)
                    _tree_eng(nc, k).tensor_add(
                        v2, v1[:, 0:4, :], v1[:, 4:8, :])
                    s3t = tmp.tile([128, 2 * CS * M], BF16, tag="s3")
                    v3 = s3t.rearrange("p (n f) -> p n f", n=2)
                    _tree_eng(nc, k).tensor_add(
                        v3, v2[:, 0:2, :], v2[:, 2:4, :])
                    nc.vector.tensor_add(s_parts[:, k, :],
                                         v3[:, 0, :], v3[:, 1, :])
                sp1 = rsmall.tile([128, DM], F32, tag="sp1")
                nc.vector.tensor_add(sp1, s_parts[:, 0, :], s_parts[:, 1, :])
                sp2 = rsmall.tile([128, DM], F32, tag="sp2")
                nc.vector.tensor_add(sp2, s_parts[:, 2, :], s_parts[:, 3, :])
                s_acc = rsmall.tile([128, DM], F32, tag="s_acc")
                nc.vector.tensor_add(s_acc, sp1, sp2)
                sg = _allreduce(tc, dram, rsmall, s_acc[:], it - 1, n_cores)
                o = _squash(tc, rsmall, sg[:], 1.0, eps_t)
                if it != _cfg["iters"][-1]:
                    o_bf = persist.tile([128, DM], BF16, tag="o_bf")
                    nc.scalar.copy(o_bf, o[:])

            nc.sync.dma_start(out=out_ap, in_=o[:])


def _ar_start_v3(tc, dram_pool, sb_pool, src, it, mh, n_cores=N_CORES):
    """AllReduce [128, DMH] f32; staging DMAs ride the (idle) gpsimd SWDGE
    queue so they never head-of-line-block the HWDGE input streams.
    Returns the SBUF dst tile (valid once the gps queue reaches its fetch)."""
    nc = tc.nc
    bin_ = dram_pool.tile([128, DMH], F32, tag=f"arin{it}{mh}")
    bout = dram_pool.tile([128, DMH], F32, tag=f"arout{it}{mh}")
    nc.gpsimd.dma_start(out=bin_[:], in_=src)
    if n_cores > 1 and not _cfg.get("no_cc"):
        nc.gpsimd.collective_compute(
            "AllReduce", mybir.AluOpType.add,
            replica_groups=[list(range(n_cores))],
            ins=[bin_.opt()], outs=[bout.opt()],
        )
    else:
        nc.gpsimd.dma_start(out=bout[:], in_=bin_[:])
    dst = sb_pool.tile([128, DMH], F32, tag=f"sg{mh}")
    nc.gpsimd.dma_start(out=dst[:], in_=bout[:])
    return dst


def _scl_chain_v3(tc, pool, ssb_f, eps_t, mh):
    """squash scale per (b, m): scl = s2/(1+s2)/sqrt(s2+eps); off the
    critical path (the raw b-update only needs ssb, scale applied after)."""
    nc = tc.nc
    sq = pool.tile([128, DMH], F32, tag=f"sq{mh}")
    nc.vector.tensor_mul(sq, ssb_f, ssb_f)
    s2h = pool.tile([128, MW], F32, tag=f"s2h{mh}")
    nc.vector.tensor_reduce(
        s2h, sq.rearrange("p (d m) -> p m d", d=D),
        axis=mybir.AxisListType.X, op=mybir.AluOpType.add)
    rt = pool.tile([128, MW], F32, tag=f"rt{mh}")
    nc.scalar.activation(rt, s2h, mybir.ActivationFunctionType.Sqrt,
                         bias=eps_t[:, 0:1])
    den = pool.tile([128, MW], F32, tag=f"den{mh}")
    nc.vector.scalar_tensor_tensor(
        out=den, in0=s2h, scalar=1.0, in1=rt,
        op0=mybir.AluOpType.add, op1=mybir.AluOpType.mult)
    rec = pool.tile([128, MW], F32, tag=f"rec{mh}")
    nc.vector.reciprocal(rec, den)
    scl = pool.tile([128, MW], F32, tag=f"scl{mh}")
    nc.vector.tensor_mul(scl, s2h, rec)
    return scl


def _bup_raw_half_v3(tc, tmp, ihb, ssb_bf, braw, mh):
    """braw[n, j] = sum_d ssb[d, j] * ih[n, d, j] for one m-half.
    The (d, m) free layout makes every d-halving a contiguous block split,
    so all tree operands are plain 3-dim [p, n, f] slices."""
    nc = tc.nc
    kk = 2 * mh
    tt = tmp.tile([128, CSR * DMH], BF16, tag="t")
    t = tt.rearrange("p (n f) -> p n f", n=CSR)
    _mul_eng(nc, kk).tensor_mul(
        t, ihb[:, :, mh, :],
        ssb_bf.unsqueeze(1).broadcast_to([128, CSR, DMH]))
    l1t = tmp.tile([128, CSR * DMH], BF16, tag="ce")
    l1 = l1t[:, 0:CSR * 8 * MW].rearrange("p (n f) -> p n f", n=CSR)
    _tree_eng(nc, kk).tensor_add(l1, t[:, :, 0:128], t[:, :, 128:256])
    l2t = tmp.tile([128, CSR * 4 * MW], BF16, tag="l2")
    l2 = l2t.rearrange("p (n f) -> p n f", n=CSR)
    _tree_eng(nc, kk).tensor_add(l2, l1[:, :, 0:64], l1[:, :, 64:128])
    l3t = tmp.tile([128, CSR * 2 * MW], BF16, tag="l3")
    l3 = l3t.rearrange("p (n f) -> p n f", n=CSR)
    _tree_eng(nc, kk).tensor_add(l3, l2[:, :, 0:32], l2[:, :, 32:64])
    nc.vector.tensor_add(braw, l3[:, :, 0:16], l3[:, :, 16:32])


def _s_half_v3(tc, tmp, rp, ihb, c_t, it, mh):
    """s[d, j] = sum_n c[n, j] * ih[n, d, j] for one m-half (f32 result)."""
    nc = tc.nc
    mhsl = slice(mh * MW, (mh + 1) * MW)
    kk = 2 * mh
    # expand c over d on the idle ACT/GPS engines: the 4-dim broadcast
    # multiply runs ~5x slower on the DVE than a dense tensor_tensor, so
    # materializing c[n, d, m] first is a large net win
    cet = tmp.tile([128, CSR * DMH], BF16, tag="ce")
    ce4 = cet.rearrange("p (n d m) -> p n d m", n=CSR, d=D)
    c_b = c_t[:, :, mhsl].unsqueeze(2).broadcast_to([128, CSR, D, MW])
    if mh == 0:
        nc.scalar.copy(ce4, c_b)
    else:
        nc.gpsimd.tensor_copy(ce4, c_b)
    t2t = tmp.tile([128, CSR * DMH], BF16, tag="t")
    t2f = t2t.rearrange("p (n f) -> p n f", n=CSR)
    _mul_eng(nc, kk).tensor_mul(
        t2f, ihb[:, :, mh, :], cet.rearrange("p (n f) -> p n f", n=CSR))
    v1t = tmp.tile([128, CSR * DMH], BF16, tag="ce")
    v1 = v1t[:, 0:CSR * 8 * MW].rearrange("p (n f) -> p n f", n=32)
    _tree_eng(nc, kk).tensor_add(v1, t2f[:, 0:32, :], t2f[:, 32:64, :])
    v2t = tmp.tile([128, CSR * 4 * MW], BF16, tag="l2")
    v2 = v2t.rearrange("p (n f) -> p n f", n=16)
    _tree_eng(nc, kk).tensor_add(v2, v1[:, 0:16, :], v1[:, 16:32, :])
    v3t = tmp.tile([128, CSR * 2 * MW], BF16, tag="l3")
    v3_ = v3t.rearrange("p (n f) -> p n f", n=8)
    _tree_eng(nc, kk).tensor_add(v3_, v2[:, 0:8, :], v2[:, 8:16, :])
    v4t = tmp.tile([128, CSR * MW], BF16, tag="l4")
    v4 = v4t.rearrange("p (n f) -> p n f", n=4)
    _tree_eng(nc, kk).tensor_add(v4, v3_[:, 0:4, :], v3_[:, 4:8, :])
    v5t = tmp.tile([128, CSR * MW // 2], BF16, tag="l5")
    v5 = v5t.rearrange("p (n f) -> p n f", n=2)
    _tree_eng(nc, kk).tensor_add(v5, v4[:, 0:2, :], v4[:, 2:4, :])
    s_half = rp.tile([128, DMH], F32, tag=f"sh{mh}")
    nc.vector.tensor_add(s_half, v5[:, 0, :], v5[:, 1, :])
    return s_half


def _body_once_v3(tc, out_ap, wtr, xtr, n_cores=N_CORES):
    nc = tc.nc
    X = mybir.AxisListType.X
    ADD = mybir.AluOpType.add

    with tc.tile_pool(name="persist", bufs=1) as persist, \
         tc.tile_pool(name="dram", bufs=1, space="DRAM") as dram, \
         tc.tile_pool(name="rsm", bufs=2) as rsm:
        ihb = persist.tile([128, NL, MH, DMH], BF16)   # 64KB/part
        eps_t = persist.tile([128, 1], F32, tag="eps")
        nc.vector.memset(eps_t, EPS)
        o_all = persist.tile([128, DM], F32, tag="o_all")

        # ---------------- einsum phase (m-half outer) ----------------
        ar_dst = [None, None]
        with tc.tile_pool(name="psum_s1", bufs=1, space="PSUM") as psum_s1, \
             tc.tile_pool(name="xt_pool", bufs=1) as xt_pool, \
             tc.tile_pool(name="wt_pool", bufs=2) as wt_pool, \
             tc.tile_pool(name="psum_mm", bufs=4, space="PSUM") as psum_mm:
            xt_all = xt_pool.tile([128, 2, NL, B], BF16)   # 32KB/part
            nc.scalar.dma_start(
                out=xt_all[:],
                in_=xtr.rearrange("(h p) n b -> p h n b", p=128),
            )
            for mh in range(MH):
                s1_ps = psum_s1.tile([128, DMH], F32, tag=f"s1_{mh}")
                for blk in range(NL // NB3):
                    wt_t = wt_pool.tile([128, 2, NB3, DMH], BF16,
                                        tag=f"wt_{blk % 2}")
                    nc.sync.dma_start(
                        out=wt_t[:],
                        in_=wtr[mh, blk].rearrange("(h p) n f -> p h n f",
                                                   p=128))
                    for j in range(NB3):
                        n = blk * NB3 + j
                        ps = psum_mm.tile([128, DMH], F32)
                        nc.tensor.matmul(ps, lhsT=xt_all[:, 0, n, :],
                                         rhs=wt_t[:, 0, j, :],
                                         start=True, stop=False)
                        nc.tensor.matmul(s1_ps, lhsT=xt_all[:, 0, n, :],
                                         rhs=wt_t[:, 0, j, :],
                                         start=(n == 0), stop=False,
                                         skip_group_check=True)
                        nc.tensor.matmul(ps, lhsT=xt_all[:, 1, n, :],
                                         rhs=wt_t[:, 1, j, :],
                                         start=False, stop=True)
                        nc.tensor.matmul(s1_ps, lhsT=xt_all[:, 1, n, :],
                                         rhs=wt_t[:, 1, j, :],
                                         start=False, stop=(n == NL - 1),
                                         skip_group_check=True)
                        # half 0 copies on DVE; half 1 on ACT so the DVE can
                        # run the first b-update pass under the h1 einsum
                        if mh == 0:
                            nc.vector.tensor_copy(ihb[:, n, mh, :], ps)
                        else:
                            nc.scalar.copy(ihb[:, n, mh, :], ps[:])
                # end of half: raw s1 partial to SBUF, AllReduce in flight
                s1sb = rsm.tile([128, DMH], F32, tag=f"s1sb{mh}")
                nc.scalar.mul(out=s1sb, in_=s1_ps[:], mul=1.0)
                ar_dst[mh] = _ar_start_v3(tc, dram, rsm, s1sb[:], 0, mh,
                                          n_cores)

        # ---------------- routing ----------------
        with tc.tile_pool(name="rp", bufs=1) as rp, \
             tc.tile_pool(name="tmp", bufs=1) as tmp:
            b_log = rp.tile([128, NL, M], F32)
            braw0 = rp.tile([128, NL, MW], F32, tag="braw0")
            braw1 = rp.tile([128, NL, MW], F32, tag="braw1")
            braws = [braw0, braw1]
            prev_dst = ar_dst
            prev_scale = 1.0 / M
            for idx, it in enumerate(_cfg["iters"]):
                first_it = idx == 0
                ssbs, ssbfs = [], []
                for mh in range(MH):
                    if prev_scale != 1.0:
                        ssb_f = rsm.tile([128, DMH], F32, tag=f"ssbf{mh}")
                        nc.vector.tensor_scalar_mul(ssb_f, prev_dst[mh][:],
                                                    prev_scale)
                    else:
                        ssb_f = prev_dst[mh]
                    ssb_bf = rsm.tile([128, DMH], BF16, tag=f"ssbb{mh}")
                    nc.vector.tensor_copy(ssb_bf, ssb_f[:])
                    ssbfs.append(ssb_f)
                    _bup_raw_half_v3(tc, tmp, ihb, ssb_bf, braws[mh], mh)
                for mh in range(MH):
                    scl = _scl_chain_v3(tc, rsm, ssbfs[mh][:], eps_t, mh)
                    mhsl = slice(mh * MW, (mh + 1) * MW)
                    scl_b = scl.unsqueeze(1).broadcast_to([128, NL, MW])
                    if first_it:
                        nc.vector.tensor_mul(b_log[:, :, mhsl], braws[mh],
                                             scl_b)
                    else:
                        bupd = rp.tile([128, NL, MW], F32, tag=f"bupd{mh}")
                        nc.vector.tensor_mul(bupd, braws[mh], scl_b)
                        nc.vector.tensor_add(b_log[:, :, mhsl],
                                             b_log[:, :, mhsl], bupd)
                # softmax over the full m (the only m-coupling point)
                e_t = rp.tile([128, NL, M], BF16, tag="e_t")
                nc.scalar.activation(e_t, b_log,
                                     mybir.ActivationFunctionType.Exp)
                zt = rsm.tile([128, NL], F32, tag="zt")
                nc.vector.tensor_reduce(zt, e_t, axis=X, op=ADD)
                rz = rsm.tile([128, NL], F32, tag="rz")
                nc.vector.reciprocal(rz, zt)
                rzb = rsm.tile([128, NL], BF16, tag="rzb")
                nc.scalar.copy(rzb, rz[:])
                c_t = e_t
                nc.vector.tensor_mul(
                    c_t, e_t, rzb.unsqueeze(2).broadcast_to([128, NL, M]))
                new_dst = []
                for mh in range(MH):
                    s_half = _s_half_v3(tc, tmp, rp, ihb, c_t, it, mh)
                    new_dst.append(_ar_start_v3(tc, dram, rsm, s_half[:],
                                                it, mh, n_cores))
                prev_dst = new_dst
                prev_scale = 1.0
            # ---------------- final output ----------------
            o_v = o_all.rearrange("p (d m) -> p d m", d=D)
            for mh in range(MH):
                if prev_scale != 1.0:
                    ssb_f = rsm.tile([128, DMH], F32, tag=f"ssbf{mh}")
                    nc.vector.tensor_scalar_mul(ssb_f, prev_dst[mh][:],
                                                prev_scale)
                else:
                    ssb_f = prev_dst[mh]
                scl = _scl_chain_v3(tc, rsm, ssb_f[:], eps_t, mh)
                mhsl = slice(mh * MW, (mh + 1) * MW)
                nc.vector.tensor_mul(
                    o_v[:, :, mhsl],
                    ssb_f.rearrange("p (d m) -> p d m", d=D),
                    scl.unsqueeze(1).broadcast_to([128, D, MW]))
            nc.sync.dma_start(out=out_ap, in_=o_all[:])


_cache = {}


def _build(n_cores=N_CORES):
    key = ("nc", n_cores, _cfg["routing"], tuple(_cfg["iters"]),
           _cfg["reps"], _cfg["mul_eng"], _cfg["tree_eng"], _cfg["ver"],
           _cfg["s1_dup"], _cfg["copy_eng"], _cfg["nb"], _cfg.get("no_cc"),
           _cfg.get("bench"))
    if key in _cache:
        return _cache[key]
    nc = bacc.Bacc("TRN2", target_bir_lowering=False, debug=False,
                   enable_asserts=True, num_devices=n_cores)
    if _cfg["ver"] == 3:
        wtr = nc.dram_tensor("wtr", [MH, NL // NB3, I, NB3, DMH], BF16,
                             kind="ExternalInput").ap()
    else:
        wtr = nc.dram_tensor("wtr", [NL // _cfg["nb"], I, _cfg["nb"], DM],
                             BF16, kind="ExternalInput").ap()
    xtr = nc.dram_tensor("xtr", [I, NL, B], BF16, kind="ExternalInput").ap()
    out = nc.dram_tensor("out", [B, DM], F32, kind="ExternalOutput").ap()
    with tile.TileContext(nc) as tc:
        _body(tc, out, wtr, xtr, n_cores)
    nc.compile()
    _cache[key] = nc
    return nc


def make_in_maps(x, W):
    """Host-side shard prep: per-core transposed bf16 views of x and W."""
    mmdt = mybir.dt.np(BF16)
    # XT[i, n, b]: one whole-slice DMA per core (16KB lines per partition)
    XT = np.ascontiguousarray(x.transpose(2, 1, 0)).astype(mmdt)
    WTbase = np.ascontiguousarray(W.transpose(1, 3, 2, 0)).astype(mmdt)
    in_maps = []
    for c in range(N_CORES):
        sl = slice(c * NL, (c + 1) * NL)
        if _cfg["ver"] == 3:
            a = WTbase[sl].reshape(NL, I, D, MH, MW)      # [n, i, d, mh, j]
            a = a.transpose(3, 0, 1, 2, 4)                # [mh, n, i, d, j]
            a = a.reshape(MH, NL // NB3, NB3, I, DMH)
            wtc = np.ascontiguousarray(a.transpose(0, 1, 3, 2, 4))
        else:
            wtc = WTbase[sl].reshape(NL, I, DM)
            nbs = _cfg["nb"]
            wtc = np.ascontiguousarray(
                wtc.reshape(NL // nbs, nbs, I, DM).transpose(0, 2, 1, 3))
        in_maps.append({
            "wtr": wtc,
            "xtr": np.ascontiguousarray(XT[:, sl, :]),
        })
    return in_maps


def kernel(x, W, _trace=False):
    x = np.asarray(x, dtype=np.float32)
    W = np.asarray(W, dtype=np.float32)
    nc = _build()
    in_maps = make_in_maps(x, W)
    res = run_bass_kernel_spmd(nc, in_maps, core_ids=list(range(N_CORES)),
                               trace=_trace)
    _cache["last_result"] = res
    # ih free layout is (d, m) -> output comes back as [B, D, M]
    return res.results[0]["out"].reshape(B, D, M).transpose(0, 2, 1).copy()


# revision 22
# speedup vs baseline: 1.7245x; 1.7245x over previous
"""CapsuleLayer (dynamic routing) Trainium2 Bass kernel.

Full inputs:  x [128, 512, 256] f32, W [32, 512, 16, 256] f32
Full output:  [128, 32, 16] f32

Sharding: split the input-capsule dim N=512 across 8 cores (64 each).
Each core computes its slice of inputs_hat = einsum('bni,mndi->bmnd')
in bf16, keeps it SBUF-resident as [b=128 part, (n_loc, d, m) free],
runs the 3 routing iterations locally (softmax over m is fully local),
and the per-core partial s = sum_n c*inputs_hat is AllReduced (256KB)
once per iteration.  W and x are each read from HBM exactly once in
aggregate (~21MB per core in bf16), the memory roofline.

v2: inputs_hat stored bf16 (2x DVE tensor_tensor mode); the segmented
reductions are balanced binary tree-adds (tensor_tensor at 2x) instead
of 1x-capped tensor_reduce; softmax exp runs on the scalar engine; DMA
layouts are host-repacked so every HBM transfer has >=2KB contiguous
lines and >=0.5MB size.
"""

import sys

sys.path.insert(0, "/opt/trn_rl_repo")

import numpy as np

import concourse.bacc as bacc
import concourse.mybir as mybir
import concourse.tile as tile
from concourse.bass_utils import run_bass_kernel_spmd

N_CORES = 8
B, N, I = 128, 512, 256
M, D = 32, 16
DM = D * M                 # ih free layout is (d, m): m innermost
NL = N // N_CORES          # 64 local input capsules per core
EPS = 1e-7
F32 = mybir.dt.float32
BF16 = mybir.dt.bfloat16

NB = 8                     # n-block size per wt DMA (may be overridden by cfg)
CS = 16                    # n-chunk size for routing passes
NCH = NL // CS
MH = 2                     # m-halves (v3): AR + squash + b-update pipelined per half
MW = M // MH               # 16 m per half
DMH = D * MW               # 256 free elems per half
NB3 = 16                   # v3 wt DMA n-block (2MB per transfer)
CSR = 64                   # v3 routing n-chunk (one chunk per m-half)

# debug/profiling knobs (defaults = full kernel)
_cfg = {
    "routing": True, "iters": (2, 3), "reps": 1,
    "mul_eng": "dve",      # leaf-multiply engine: dve | gps | split
    "tree_eng": "dve",     # tree-add engine: dve | split
    "s1_dup": True,        # accumulate iter-1 s via duplicated PE matmuls
    "copy_eng": "dve",     # inputs_hat PSUM->SBUF copy engine: dve | act
    "nb": NB,              # n-block size per wt DMA (8 -> 2MB, 16 -> 4MB)
    "ver": 3,              # 2 = monolithic-m body, 3 = m-half pipelined body
}

import json as _json
import os as _os

_cfg.update(_json.loads(_os.environ.get("KCFG", "{}")))
if "iters" in _cfg:
    _cfg["iters"] = tuple(_cfg["iters"])


def _squash(tc, pool, s_src, scale_pre, eps_t):
    """o = squash(s) over d; s layout [128, (d, m)]. Returns o tile (f32)."""
    nc = tc.nc
    ssb = pool.tile([128, DM], F32, tag="ssb")
    nc.scalar.mul(out=ssb, in_=s_src, mul=scale_pre)  # copy (+scale) to SBUF
    sq = pool.tile([128, DM], F32, tag="sq")
    nc.vector.tensor_mul(sq, ssb, ssb)
    s2 = pool.tile([128, M], F32, tag="s2")
    nc.vector.tensor_reduce(
        s2, sq.rearrange("p (d m) -> p m d", d=D),
        axis=mybir.AxisListType.X, op=mybir.AluOpType.add,
    )
    rt = pool.tile([128, M], F32, tag="rt")
    nc.scalar.activation(rt, s2, mybir.ActivationFunctionType.Sqrt,
                         bias=eps_t[:, 0:1])
    one_p = pool.tile([128, M], F32, tag="one_p")
    nc.vector.tensor_scalar_add(one_p, s2, 1.0)
    den = pool.tile([128, M], F32, tag="den")
    nc.vector.tensor_mul(den, one_p, rt)
    rec = pool.tile([128, M], F32, tag="rec")
    nc.vector.reciprocal(rec, den)
    scl = pool.tile([128, M], F32, tag="scl")
    nc.vector.tensor_mul(scl, s2, rec)      # scale = s2/(1+s2)/sqrt(s2+eps)
    o = pool.tile([128, DM], F32, tag="o")
    nc.vector.tensor_mul(
        o.rearrange("p (d m) -> p d m", d=D),
        ssb.rearrange("p (d m) -> p d m", d=D),
        scl.unsqueeze(1).broadcast_to([128, D, M]),
    )
    return o


def _allreduce(tc, dram_pool, sb_pool, src, idx, n_cores=N_CORES):
    """AllReduce [128, DM] f32 across the cores. Returns SBUF tile."""
    nc = tc.nc
    bin_ = dram_pool.tile([128, DM], F32, tag=f"arin{idx}")
    bout = dram_pool.tile([128, DM], F32, tag=f"arout{idx}")
    nc.sync.dma_start(out=bin_[:], in_=src)
    if n_cores > 1 and not _cfg.get("no_cc"):
        nc.gpsimd.collective_compute(
            "AllReduce", mybir.AluOpType.add,
            replica_groups=[list(range(n_cores))],
            ins=[bin_.opt()], outs=[bout.opt()],
        )
    else:
        nc.sync.dma_start(out=bout[:], in_=bin_[:])  # sim stand-in
    dst = sb_pool.tile([128, DM], F32, tag="sglob")
    nc.sync.dma_start(out=dst[:], in_=bout[:])
    return dst


def _mul_eng(nc, k):
    return {"dve": nc.vector, "gps": nc.gpsimd,
            "split": nc.gpsimd if k % 4 == 3 else nc.vector}[_cfg["mul_eng"]]


def _tree_eng(nc, k):
    return {"dve": nc.vector,
            "split": nc.gpsimd if k % 4 == 3 else nc.vector}[_cfg["tree_eng"]]


def _bench_body(tc, out_ap):
    """Microbenchmark: N ops of one pattern per rep (cfg bench=pattern,N)."""
    nc = tc.nc
    pat, nops = _cfg["bench"].split(",")
    nops = int(nops)
    with tc.tile_pool(name="bp", bufs=1) as bp:
        FD = 16384
        a = bp.tile([128, FD], BF16, tag="a")
        b = bp.tile([128, FD], BF16, tag="b")
        o = bp.tile([128, FD], BF16, tag="o")
        af = bp.tile([128, FD // 2], F32, tag="af")
        bf = bp.tile([128, FD // 2], F32, tag="bf")
        of = bp.tile([128, FD // 2], F32, tag="of")
        sm = bp.tile([128, DMH], BF16, tag="sm")
        red = bp.tile([128, 64], F32, tag="red")
        nc.vector.memset(a, 0.5)
        nc.vector.memset(b, 0.25)
        nc.vector.memset(af, 0.5)
        nc.vector.memset(bf, 0.25)
        nc.vector.memset(sm, 0.5)
        for i in range(nops):
            if pat == "tt_dense":
                nc.vector.tensor_mul(o, a, b)
            elif pat == "tt_dense_f32":
                nc.vector.tensor_mul(of, af, bf)
            elif pat == "tt_bcast":
                # [p, 64, 256] with in1 broadcast over n (the bup leaf shape)
                nc.vector.tensor_mul(
                    o.rearrange("p (n f) -> p n f", n=64),
                    a.rearrange("p (n f) -> p n f", n=64),
                    sm.unsqueeze(1).broadcast_to([128, 64, DMH]))
            elif pat == "tt_bcast4":
                # 4-dim strided mul (the s-step leaf shape):
                # in1 = c[p, n, m] broadcast over d
                cm = b.rearrange("p (n m x) -> p n m x", n=64, m=16)[:, :, :, 0]
                nc.vector.tensor_mul(
                    o.rearrange("p (n d m) -> p n d m", n=64, d=D),
                    a.rearrange("p (n d m) -> p n d m", n=64, d=D),
                    cm.unsqueeze(2).broadcast_to([128, 64, D, 16]))
            elif pat == "tt_tree":
                # 4-dim strided tree add (the d-tree shape)
                av = a.rearrange("p (n d m) -> p n d m", n=64, d=D)
                nc.vector.tensor_add(
                    o.rearrange("p (n d m) -> p n d m", n=128, d=8)[:, 0:64],
                    av[:, :, 0:8, :], av[:, :, 8:16, :])
            elif pat == "reduce":
                nc.vector.tensor_reduce(
                    red, a.rearrange("p (n f) -> p n f", n=64),
                    axis=mybir.AxisListType.X, op=mybir.AluOpType.add)
            elif pat == "gps_tt":
                nc.gpsimd.tensor_mul(o, a, b)
            elif pat == "act_copy":
                nc.scalar.copy(o, a[:])
            elif pat == "act_expand":
                cm = b.rearrange("p (n m x) -> p n m x", n=64, m=16)[:, :, :, 0]
                nc.scalar.copy(
                    o.rearrange("p (n d m) -> p n d m", n=64, d=D),
                    cm.unsqueeze(2).broadcast_to([128, 64, D, 16]))
            elif pat == "gps_expand":
                cm = b.rearrange("p (n m x) -> p n m x", n=64, m=16)[:, :, :, 0]
                nc.gpsimd.tensor_copy(
                    o.rearrange("p (n d m) -> p n d m", n=64, d=D),
                    cm.unsqueeze(2).broadcast_to([128, 64, D, 16]))
            elif pat == "dve_expand":
                cm = b.rearrange("p (n m x) -> p n m x", n=64, m=16)[:, :, :, 0]
                nc.vector.tensor_copy(
                    o.rearrange("p (n d m) -> p n d m", n=64, d=D),
                    cm.unsqueeze(2).broadcast_to([128, 64, D, 16]))
            elif pat == "dve_expand32":
                # expand via 32-elem-run copy: in innermost run = 32 (2 m-rows)
                cm2 = b.rearrange("p (n m x) -> p n m x", n=32, m=32)[:, :, :, 0]
                nc.vector.tensor_copy(
                    o.rearrange("p (n d m) -> p n d m", n=32, d=D),
                    cm2.unsqueeze(2).broadcast_to([128, 32, D, 32]))
        nc.sync.dma_start(out=out_ap, in_=af[:, 0:DM])


def _body(tc, out_ap, wtr, xtr, n_cores=N_CORES):
    if _cfg.get("bench"):
        for _rep in range(_cfg.get("reps", 1)):
            _bench_body(tc, out_ap)
        return
    body = _body_once_v3 if _cfg["ver"] == 3 else _body_once
    for _rep in range(_cfg.get("reps", 1)):
        body(tc, out_ap, wtr, xtr, n_cores)


def _body_once(tc, out_ap, wtr, xtr, n_cores=N_CORES):
    nc = tc.nc
    X = mybir.AxisListType.X
    ADD = mybir.AluOpType.add

    with tc.tile_pool(name="persist", bufs=1) as persist, \
         tc.tile_pool(name="dram", bufs=1, space="DRAM") as dram:
        ihb = persist.tile([128, NL, DM], BF16)    # inputs_hat bf16, 64KB/part
        eps_t = persist.tile([128, 1], F32, tag="eps")
        nc.vector.memset(eps_t, EPS)

        # ---------------- einsum phase ----------------
        with tc.tile_pool(name="psum_s1", bufs=1, space="PSUM") as psum_s1:
            if _cfg["s1_dup"]:
                s1_ps = psum_s1.tile([128, DM], F32)  # sum_n ih (PE-accum)
            with tc.tile_pool(name="xt_pool", bufs=1) as xt_pool, \
                 tc.tile_pool(name="wt_pool", bufs=2) as wt_pool, \
                 tc.tile_pool(name="psum_mm", bufs=4, space="PSUM") as psum_mm:
                nbs = _cfg["nb"]
                xt_all = xt_pool.tile([128, 2, NL, B], BF16)   # 32KB/part
                xt_eng = nc.scalar if _cfg["copy_eng"] == "act" else nc.sync
                xt_eng.dma_start(
                    out=xt_all[:],
                    in_=xtr.rearrange("(h p) n b -> p h n b", p=128),
                )
                for nb in range(NL // nbs):
                    wt_t = wt_pool.tile([128, 2, nbs, DM], BF16,
                                        tag=f"wt_{nb % 2}")
                    if _cfg["copy_eng"] == "act":
                        # ACT queue is kept free for the ihb copies; the wt
                        # stream is serial on the sync ring (it is the bulk
                        # HBM stream and back-to-back regardless)
                        dma_eng = nc.sync
                    else:
                        dma_eng = nc.sync if nb % 2 == 0 else nc.scalar
                    dma_eng.dma_start(
                        out=wt_t[:],
                        in_=wtr[nb].rearrange("(h p) n m -> p h n m", p=128))
                    for j in range(nbs):
                        n = nb * nbs + j
                        ps = psum_mm.tile([128, DM], F32)
                        nc.tensor.matmul(ps, lhsT=xt_all[:, 0, n, :],
                                         rhs=wt_t[:, 0, j, :],
                                         start=True, stop=False)
                        if _cfg["s1_dup"]:
                            nc.tensor.matmul(s1_ps, lhsT=xt_all[:, 0, n, :],
                                             rhs=wt_t[:, 0, j, :],
                                             start=(n == 0), stop=False,
                                             skip_group_check=True)
                        nc.tensor.matmul(ps, lhsT=xt_all[:, 1, n, :],
                                         rhs=wt_t[:, 1, j, :],
                                         start=False, stop=True)
                        if _cfg["s1_dup"]:
                            nc.tensor.matmul(s1_ps, lhsT=xt_all[:, 1, n, :],
                                             rhs=wt_t[:, 1, j, :],
                                             start=False, stop=(n == NL - 1),
                                             skip_group_check=True)
                        if _cfg["copy_eng"] == "act":
                            nc.scalar.copy(ihb[:, n, :], ps[:])
                        else:
                            nc.vector.tensor_copy(ihb[:, n, :], ps)

            # -------- iteration 1 (uniform c): s1 = sum_n ih / M --------
            with tc.tile_pool(name="rs0", bufs=1) as rs0:
                s1_sb = rs0.tile([128, DM], F32, tag="s1_sb")
                if _cfg["s1_dup"]:
                    nc.scalar.mul(out=s1_sb, in_=s1_ps[:], mul=1.0 / M)
                else:
                    # binary tree over n on DVE (bf16 2x mode)
                    w1 = rs0.tile([128, 32, DM], BF16, tag="w1")
                    nc.vector.tensor_add(w1, ihb[:, 0:32, :], ihb[:, 32:64, :])
                    w2 = rs0.tile([128, 16, DM], BF16, tag="w2")
                    nc.vector.tensor_add(w2, w1[:, 0:16, :], w1[:, 16:32, :])
                    w3 = rs0.tile([128, 8, DM], BF16, tag="w3")
                    nc.vector.tensor_add(w3, w2[:, 0:8, :], w2[:, 8:16, :])
                    w4 = rs0.tile([128, 4, DM], BF16, tag="w4")
                    nc.vector.tensor_add(w4, w3[:, 0:4, :], w3[:, 4:8, :])
                    w5 = rs0.tile([128, 2, DM], BF16, tag="w5")
                    nc.vector.tensor_add(w5, w4[:, 0:2, :], w4[:, 2:4, :])
                    s1f = rs0.tile([128, DM], F32, tag="s1f")
                    nc.vector.tensor_add(s1f, w5[:, 0, :], w5[:, 1, :])
                    nc.scalar.mul(out=s1_sb, in_=s1f[:], mul=1.0 / M)
                s1g = _allreduce(tc, dram, rs0, s1_sb[:], 0, n_cores)
                o = _squash(tc, persist, s1g[:], 1.0, eps_t)
                o_bf = persist.tile([128, DM], BF16, tag="o_bf")
                nc.scalar.copy(o_bf, o[:])

        if not _cfg["routing"]:
            nc.sync.dma_start(out=out_ap, in_=o[:])
            return

        # ---------------- routing iterations 2..3 ----------------
        with tc.tile_pool(name="rp", bufs=1) as rp, \
             tc.tile_pool(name="rsmall", bufs=2) as rsmall, \
             tc.tile_pool(name="tmp", bufs=2) as tmp:
            b_log = rp.tile([128, NL, M], F32)     # routing logits
            for it in _cfg["iters"]:
                first_it = it == _cfg["iters"][0]
                # ---- b-update: b_log (+)= sum_d o * ih (binary tree) ----
                for k in range(NCH):
                    ksl = slice(k * CS, (k + 1) * CS)
                    t = tmp.tile([128, CS, DM], BF16, tag="t")
                    _mul_eng(nc, k).tensor_mul(
                        t, ihb[:, ksl, :],
                        o_bf.unsqueeze(1).broadcast_to([128, CS, DM]),
                    )
                    tv = t.rearrange("p n (d m) -> p n d m", d=D)
                    s1t = tmp.tile([128, 8 * CS * M], BF16, tag="s1")
                    l1 = s1t.rearrange("p (n d m) -> p n d m", n=CS, d=8)
                    _tree_eng(nc, k).tensor_add(
                        l1, tv[:, :, 0:8, :], tv[:, :, 8:16, :])
                    s2t = tmp.tile([128, 4 * CS * M], BF16, tag="s2")
                    l2 = s2t.rearrange("p (n d m) -> p n d m", n=CS, d=4)
                    _tree_eng(nc, k).tensor_add(
                        l2, l1[:, :, 0:4, :], l1[:, :, 4:8, :])
                    s3t = tmp.tile([128, 2 * CS * M], BF16, tag="s3")
                    l3 = s3t.rearrange("p (n d m) -> p n d m", n=CS, d=2)
                    _tree_eng(nc, k).tensor_add(
                        l3, l2[:, :, 0:2, :], l2[:, :, 2:4, :])
                    if first_it:
                        nc.vector.tensor_add(b_log[:, ksl, :],
                                             l3[:, :, 0, :], l3[:, :, 1, :])
                    else:
                        bup = tmp.tile([128, CS, M], F32, tag="bup")
                        nc.vector.tensor_add(bup, l3[:, :, 0, :],
                                             l3[:, :, 1, :])
                        nc.vector.tensor_add(b_log[:, ksl, :],
                                             b_log[:, ksl, :], bup)
                # ---- softmax over m (innermost free dim) ----
                e_t = rp.tile([128, NL, M], BF16, tag="e_t")
                nc.scalar.activation(e_t, b_log,
                                     mybir.ActivationFunctionType.Exp)
                zt = rsmall.tile([128, NL], F32, tag="zt")
                nc.vector.tensor_reduce(zt, e_t, axis=X, op=ADD)
                rz = rsmall.tile([128, NL], F32, tag="rz")
                nc.vector.reciprocal(rz, zt)
                rzb = rsmall.tile([128, NL], BF16, tag="rzb")
                nc.scalar.copy(rzb, rz[:])
                c_t = e_t    # normalize in place: c = e * (1/Z)
                nc.vector.tensor_mul(
                    c_t, e_t, rzb.unsqueeze(2).broadcast_to([128, NL, M]))
                # ---- s-step: s = sum_n c * ih (binary tree over n) ----
                s_parts = rp.tile([128, NCH, DM], F32, tag="s_parts")
                for k in range(NCH):
                    ksl = slice(k * CS, (k + 1) * CS)
                    t2 = tmp.tile([128, CS, DM], BF16, tag="t")
                    _mul_eng(nc, k).tensor_mul(
                        t2.rearrange("p n (d m) -> p n d m", d=D),
                        ihb[:, ksl, :].rearrange("p n (d m) -> p n d m", d=D),
                        c_t[:, ksl, :].unsqueeze(2).broadcast_to(
                            [128, CS, D, M]),
                    )
                    s1t = tmp.tile([128, 8 * CS * M], BF16, tag="s1")
                    v1 = s1t.rearrange("p (n f) -> p n f", n=8)
                    _tree_eng(nc, k).tensor_add(
                        v1, t2[:, 0:8, :], t2[:, 8:16, :])
                    s2t = tmp.tile([128, 4 * CS * M], BF16, tag="s2")
                    v2 = s2t.rearrange("p (n f) -> p n f", n=4)
                    _tree_eng(nc, k).tensor_add(
                        v2, v1[:, 0:4, :], v1[:, 4:8, :])
                    s3t = tmp.tile([128, 2 * CS * M], BF16, tag="s3")
                    v3 = s3t.rearrange("p (n f) -> p n f", n=2)
                    _tree_eng(nc, k).tensor_add(
                        v3, v2[:, 0:2, :], v2[:, 2:4, :])
                    nc.vector.tensor_add(s_parts[:, k, :],
                                         v3[:, 0, :], v3[:, 1, :])
                sp1 = rsmall.tile([128, DM], F32, tag="sp1")
                nc.vector.tensor_add(sp1, s_parts[:, 0, :], s_parts[:, 1, :])
                sp2 = rsmall.tile([128, DM], F32, tag="sp2")
                nc.vector.tensor_add(sp2, s_parts[:, 2, :], s_parts[:, 3, :])
                s_acc = rsmall.tile([128, DM], F32, tag="s_acc")
                nc.vector.tensor_add(s_acc, sp1, sp2)
                sg = _allreduce(tc, dram, rsmall, s_acc[:], it - 1, n_cores)
                o = _squash(tc, rsmall, sg[:], 1.0, eps_t)
                if it != _cfg["iters"][-1]:
                    o_bf = persist.tile([128, DM], BF16, tag="o_bf")
                    nc.scalar.copy(o_bf, o[:])

            nc.sync.dma_start(out=out_ap, in_=o[:])


def _ar_start_v3(tc, dram_pool, sb_pool, src, it, mh, n_cores=N_CORES):
    """AllReduce [128, DMH] f32; staging DMAs ride the (idle) gpsimd SWDGE
    queue so they never head-of-line-block the HWDGE input streams.
    Returns the SBUF dst tile (valid once the gps queue reaches its fetch)."""
    nc = tc.nc
    bin_ = dram_pool.tile([128, DMH], F32, tag=f"arin{it}{mh}")
    bout = dram_pool.tile([128, DMH], F32, tag=f"arout{it}{mh}")
    nc.gpsimd.dma_start(out=bin_[:], in_=src)
    if n_cores > 1 and not _cfg.get("no_cc"):
        nc.gpsimd.collective_compute(
            "AllReduce", mybir.AluOpType.add,
            replica_groups=[list(range(n_cores))],
            ins=[bin_.opt()], outs=[bout.opt()],
        )
    else:
        nc.gpsimd.dma_start(out=bout[:], in_=bin_[:])
    dst = sb_pool.tile([128, DMH], F32, tag=f"sg{mh}")
    nc.gpsimd.dma_start(out=dst[:], in_=bout[:])
    return dst


def _scl_chain_v3(tc, pool, ssb_f, eps_t, mh):
    """squash scale per (b, m): scl = s2/(1+s2)/sqrt(s2+eps); off the
    critical path (the raw b-update only needs ssb, scale applied after)."""
    nc = tc.nc
    sq = pool.tile([128, DMH], F32, tag=f"sq{mh}")
    nc.vector.tensor_mul(sq, ssb_f, ssb_f)
    s2h = pool.tile([128, MW], F32, tag=f"s2h{mh}")
    nc.vector.tensor_reduce(
        s2h, sq.rearrange("p (d m) -> p m d", d=D),
        axis=mybir.AxisListType.X, op=mybir.AluOpType.add)
    rt = pool.tile([128, MW], F32, tag=f"rt{mh}")
    nc.scalar.activation(rt, s2h, mybir.ActivationFunctionType.Sqrt,
                         bias=eps_t[:, 0:1])
    den = pool.tile([128, MW], F32, tag=f"den{mh}")
    nc.vector.scalar_tensor_tensor(
        out=den, in0=s2h, scalar=1.0, in1=rt,
        op0=mybir.AluOpType.add, op1=mybir.AluOpType.mult)
    rec = pool.tile([128, MW], F32, tag=f"rec{mh}")
    nc.vector.reciprocal(rec, den)
    scl = pool.tile([128, MW], F32, tag=f"scl{mh}")
    nc.vector.tensor_mul(scl, s2h, rec)
    return scl


def _bup_raw_half_v3(tc, tmp, ihb, ssb_bf, braw, mh):
    """braw[n, j] = sum_d ssb[d, j] * ih[n, d, j] for one m-half.
    The (d, m) free layout makes every d-halving a contiguous block split,
    so all tree operands are plain 3-dim [p, n, f] slices."""
    nc = tc.nc
    kk = 2 * mh
    tt = tmp.tile([128, CSR * DMH], BF16, tag="t")
    t = tt.rearrange("p (n f) -> p n f", n=CSR)
    _mul_eng(nc, kk).tensor_mul(
        t, ihb[:, :, mh, :],
        ssb_bf.unsqueeze(1).broadcast_to([128, CSR, DMH]))
    l1t = tmp.tile([128, CSR * DMH], BF16, tag="ce")
    l1 = l1t[:, 0:CSR * 8 * MW].rearrange("p (n f) -> p n f", n=CSR)
    _tree_eng(nc, kk).tensor_add(l1, t[:, :, 0:128], t[:, :, 128:256])
    l2t = tmp.tile([128, CSR * 4 * MW], BF16, tag="l2")
    l2 = l2t.rearrange("p (n f) -> p n f", n=CSR)
    _tree_eng(nc, kk).tensor_add(l2, l1[:, :, 0:64], l1[:, :, 64:128])
    l3t = tmp.tile([128, CSR * 2 * MW], BF16, tag="l3")
    l3 = l3t.rearrange("p (n f) -> p n f", n=CSR)
    _tree_eng(nc, kk).tensor_add(l3, l2[:, :, 0:32], l2[:, :, 32:64])
    nc.vector.tensor_add(braw, l3[:, :, 0:16], l3[:, :, 16:32])


def _s_half_v3(tc, tmp, rp, ihb, c_t, it, mh):
    """s[d, j] = sum_n c[n, j] * ih[n, d, j] for one m-half (f32 result)."""
    nc = tc.nc
    mhsl = slice(mh * MW, (mh + 1) * MW)
    kk = 2 * mh
    # The 0-stride d-broadcast multiply is ~5x slower on the DVE than a
    # dense tensor_tensor, so expand c across d first via log-doubling:
    # with d outer of (d, m) every doubling is a contiguous-run copy.
    cet = tmp.tile([128, CSR * DMH], BF16, tag="ce")
    ce = cet.rearrange("p (n f) -> p n f", n=CSR)
    nc.vector.tensor_copy(ce[:, :, 0:MW], c_t[:, :, mhsl])
    nc.vector.tensor_copy(ce[:, :, MW:2 * MW], ce[:, :, 0:MW])
    nc.vector.tensor_copy(ce[:, :, 2 * MW:4 * MW], ce[:, :, 0:2 * MW])
    nc.vector.tensor_copy(ce[:, :, 4 * MW:8 * MW], ce[:, :, 0:4 * MW])
    nc.vector.tensor_copy(ce[:, :, 8 * MW:16 * MW], ce[:, :, 0:8 * MW])
    t2t = tmp.tile([128, CSR * DMH], BF16, tag="t")
    t2f = t2t.rearrange("p (n f) -> p n f", n=CSR)
    _mul_eng(nc, kk).tensor_mul(t2f, ihb[:, :, mh, :], ce)
    v1t = tmp.tile([128, CSR * DMH], BF16, tag="ce")
    v1 = v1t[:, 0:CSR * 8 * MW].rearrange("p (n f) -> p n f", n=32)
    _tree_eng(nc, kk).tensor_add(v1, t2f[:, 0:32, :], t2f[:, 32:64, :])
    v2t = tmp.tile([128, CSR * 4 * MW], BF16, tag="l2")
    v2 = v2t.rearrange("p (n f) -> p n f", n=16)
    _tree_eng(nc, kk).tensor_add(v2, v1[:, 0:16, :], v1[:, 16:32, :])
    v3t = tmp.tile([128, CSR * 2 * MW], BF16, tag="l3")
    v3_ = v3t.rearrange("p (n f) -> p n f", n=8)
    _tree_eng(nc, kk).tensor_add(v3_, v2[:, 0:8, :], v2[:, 8:16, :])
    v4t = tmp.tile([128, CSR * MW], BF16, tag="l4")
    v4 = v4t.rearrange("p (n f) -> p n f", n=4)
    _tree_eng(nc, kk).tensor_add(v4, v3_[:, 0:4, :], v3_[:, 4:8, :])
    v5t = tmp.tile([128, CSR * MW // 2], BF16, tag="l5")
    v5 = v5t.rearrange("p (n f) -> p n f", n=2)
    _tree_eng(nc, kk).tensor_add(v5, v4[:, 0:2, :], v4[:, 2:4, :])
    s_half = rp.tile([128, DMH], F32, tag=f"sh{mh}")
    nc.vector.tensor_add(s_half, v5[:, 0, :], v5[:, 1, :])
    return s_half


def _body_once_v3(tc, out_ap, wtr, xtr, n_cores=N_CORES):
    nc = tc.nc
    X = mybir.AxisListType.X
    ADD = mybir.AluOpType.add

    with tc.tile_pool(name="persist", bufs=1) as persist, \
         tc.tile_pool(name="dram", bufs=1, space="DRAM") as dram, \
         tc.tile_pool(name="rsm", bufs=2) as rsm:
        ihb = persist.tile([128, NL, MH, DMH], BF16)   # 64KB/part
        eps_t = persist.tile([128, 1], F32, tag="eps")
        nc.vector.memset(eps_t, EPS)
        o_all = persist.tile([128, DM], F32, tag="o_all")

        # ---------------- einsum phase (m-half outer) ----------------
        ar_dst = [None, None]
        with tc.tile_pool(name="psum_s1", bufs=1, space="PSUM") as psum_s1, \
             tc.tile_pool(name="xt_pool", bufs=1) as xt_pool, \
             tc.tile_pool(name="wt_pool", bufs=2) as wt_pool, \
             tc.tile_pool(name="psum_mm", bufs=4, space="PSUM") as psum_mm:
            xt_all = xt_pool.tile([128, 2, NL, B], BF16)   # 32KB/part
            nc.scalar.dma_start(
                out=xt_all[:],
                in_=xtr.rearrange("(h p) n b -> p h n b", p=128),
            )
            for mh in range(MH):
                s1_ps = psum_s1.tile([128, DMH], F32, tag=f"s1_{mh}")
                for blk in range(NL // NB3):
                    wt_t = wt_pool.tile([128, 2, NB3, DMH], BF16,
                                        tag=f"wt_{blk % 2}")
                    nc.sync.dma_start(
                        out=wt_t[:],
                        in_=wtr[mh, blk].rearrange("(h p) n f -> p h n f",
                                                   p=128))
                    for j in range(NB3):
                        n = blk * NB3 + j
                        ps = psum_mm.tile([128, DMH], F32)
                        nc.tensor.matmul(ps, lhsT=xt_all[:, 0, n, :],
                                         rhs=wt_t[:, 0, j, :],
                                         start=True, stop=False)
                        nc.tensor.matmul(s1_ps, lhsT=xt_all[:, 0, n, :],
                                         rhs=wt_t[:, 0, j, :],
                                         start=(n == 0), stop=False,
                                         skip_group_check=True)
                        nc.tensor.matmul(ps, lhsT=xt_all[:, 1, n, :],
                                         rhs=wt_t[:, 1, j, :],
                                         start=False, stop=True)
                        nc.tensor.matmul(s1_ps, lhsT=xt_all[:, 1, n, :],
                                         rhs=wt_t[:, 1, j, :],
                                         start=False, stop=(n == NL - 1),
                                         skip_group_check=True)
                        # half 0 copies on DVE; half 1 on ACT so the DVE can
                        # run the first b-update pass under the h1 einsum
                        if mh == 0:
                            nc.vector.tensor_copy(ihb[:, n, mh, :], ps)
                        else:
                            nc.scalar.copy(ihb[:, n, mh, :], ps[:])
                # end of half: raw s1 partial to SBUF, AllReduce in flight
                s1sb = rsm.tile([128, DMH], F32, tag=f"s1sb{mh}")
                nc.scalar.mul(out=s1sb, in_=s1_ps[:], mul=1.0)
                ar_dst[mh] = _ar_start_v3(tc, dram, rsm, s1sb[:], 0, mh,
                                          n_cores)

        # ---------------- routing ----------------
        with tc.tile_pool(name="rp", bufs=1) as rp, \
             tc.tile_pool(name="tmp", bufs=1) as tmp:
            b_log = rp.tile([128, NL, M], F32)
            braw0 = rp.tile([128, NL, MW], F32, tag="braw0")
            braw1 = rp.tile([128, NL, MW], F32, tag="braw1")
            braws = [braw0, braw1]
            prev_dst = ar_dst
            prev_scale = 1.0 / M
            for idx, it in enumerate(_cfg["iters"]):
                first_it = idx == 0
                ssbs, ssbfs = [], []
                for mh in range(MH):
                    if prev_scale != 1.0:
                        ssb_f = rsm.tile([128, DMH], F32, tag=f"ssbf{mh}")
                        nc.vector.tensor_scalar_mul(ssb_f, prev_dst[mh][:],
                                                    prev_scale)
                    else:
                        ssb_f = prev_dst[mh]
                    ssb_bf = rsm.tile([128, DMH], BF16, tag=f"ssbb{mh}")
                    nc.vector.tensor_copy(ssb_bf, ssb_f[:])
                    ssbfs.append(ssb_f)
                    _bup_raw_half_v3(tc, tmp, ihb, ssb_bf, braws[mh], mh)
                for mh in range(MH):
                    scl = _scl_chain_v3(tc, rsm, ssbfs[mh][:], eps_t, mh)
                    mhsl = slice(mh * MW, (mh + 1) * MW)
                    scl_b = scl.unsqueeze(1).broadcast_to([128, NL, MW])
                    if first_it:
                        nc.vector.tensor_mul(b_log[:, :, mhsl], braws[mh],
                                             scl_b)
                    else:
                        bupd = rp.tile([128, NL, MW], F32, tag=f"bupd{mh}")
                        nc.vector.tensor_mul(bupd, braws[mh], scl_b)
                        nc.vector.tensor_add(b_log[:, :, mhsl],
                                             b_log[:, :, mhsl], bupd)
                # softmax over the full m (the only m-coupling point)
                e_t = rp.tile([128, NL, M], BF16, tag="e_t")
                nc.scalar.activation(e_t, b_log,
                                     mybir.ActivationFunctionType.Exp)
                zt = rsm.tile([128, NL], F32, tag="zt")
                nc.vector.tensor_reduce(zt, e_t, axis=X, op=ADD)
                rz = rsm.tile([128, NL], F32, tag="rz")
                nc.vector.reciprocal(rz, zt)
                rzb = rsm.tile([128, NL], BF16, tag="rzb")
                nc.scalar.copy(rzb, rz[:])
                c_t = e_t
                nc.vector.tensor_mul(
                    c_t, e_t, rzb.unsqueeze(2).broadcast_to([128, NL, M]))
                new_dst = []
                for mh in range(MH):
                    s_half = _s_half_v3(tc, tmp, rp, ihb, c_t, it, mh)
                    new_dst.append(_ar_start_v3(tc, dram, rsm, s_half[:],
                                                it, mh, n_cores))
                prev_dst = new_dst
                prev_scale = 1.0
            # ---------------- final output ----------------
            o_v = o_all.rearrange("p (d m) -> p d m", d=D)
            for mh in range(MH):
                if prev_scale != 1.0:
                    ssb_f = rsm.tile([128, DMH], F32, tag=f"ssbf{mh}")
                    nc.vector.tensor_scalar_mul(ssb_f, prev_dst[mh][:],
                                                prev_scale)
                else:
                    ssb_f = prev_dst[mh]
                scl = _scl_chain_v3(tc, rsm, ssb_f[:], eps_t, mh)
                mhsl = slice(mh * MW, (mh + 1) * MW)
                nc.vector.tensor_mul(
                    o_v[:, :, mhsl],
                    ssb_f.rearrange("p (d m) -> p d m", d=D),
                    scl.unsqueeze(1).broadcast_to([128, D, MW]))
            nc.sync.dma_start(out=out_ap, in_=o_all[:])


_cache = {}


def _build(n_cores=N_CORES):
    key = ("nc", n_cores, _cfg["routing"], tuple(_cfg["iters"]),
           _cfg["reps"], _cfg["mul_eng"], _cfg["tree_eng"], _cfg["ver"],
           _cfg["s1_dup"], _cfg["copy_eng"], _cfg["nb"], _cfg.get("no_cc"),
           _cfg.get("bench"))
    if key in _cache:
        return _cache[key]
    nc = bacc.Bacc("TRN2", target_bir_lowering=False, debug=False,
                   enable_asserts=True, num_devices=n_cores)
    if _cfg["ver"] == 3:
        wtr = nc.dram_tensor("wtr", [MH, NL // NB3, I, NB3, DMH], BF16,
                             kind="ExternalInput").ap()
    else:
        wtr = nc.dram_tensor("wtr", [NL // _cfg["nb"], I, _cfg["nb"], DM],
                             BF16, kind="ExternalInput").ap()
    xtr = nc.dram_tensor("xtr", [I, NL, B], BF16, kind="ExternalInput").ap()
    out = nc.dram_tensor("out", [B, DM], F32, kind="ExternalOutput").ap()
    with tile.TileContext(nc) as tc:
        _body(tc, out, wtr, xtr, n_cores)
    nc.compile()
    _cache[key] = nc
    return nc


def make_in_maps(x, W):
    """Host-side shard prep: per-core transposed bf16 views of x and W."""
    mmdt = mybir.dt.np(BF16)
    # XT[i, n, b]: one whole-slice DMA per core (16KB lines per partition)
    XT = np.ascontiguousarray(x.transpose(2, 1, 0)).astype(mmdt)
    WTbase = np.ascontiguousarray(W.transpose(1, 3, 2, 0)).astype(mmdt)
    in_maps = []
    for c in range(N_CORES):
        sl = slice(c * NL, (c + 1) * NL)
        if _cfg["ver"] == 3:
            a = WTbase[sl].reshape(NL, I, D, MH, MW)      # [n, i, d, mh, j]
            a = a.transpose(3, 0, 1, 2, 4)                # [mh, n, i, d, j]
            a = a.reshape(MH, NL // NB3, NB3, I, DMH)
            wtc = np.ascontiguousarray(a.transpose(0, 1, 3, 2, 4))
        else:
            wtc = WTbase[sl].reshape(NL, I, DM)
            nbs = _cfg["nb"]
            wtc = np.ascontiguousarray(
                wtc.reshape(NL // nbs, nbs, I, DM).transpose(0, 2, 1, 3))
        in_maps.append({
            "wtr": wtc,
            "xtr": np.ascontiguousarray(XT[:, sl, :]),
        })
    return in_maps


def kernel(x, W, _trace=False):
    x = np.asarray(x, dtype=np.float32)
    W = np.asarray(W, dtype=np.float32)
    nc = _build()
    in_maps = make_in_maps(x, W)
    res = run_bass_kernel_spmd(nc, in_maps, core_ids=list(range(N_CORES)),
                               trace=_trace)
    _cache["last_result"] = res
    # ih free layout is (d, m) -> output comes back as [B, D, M]
    return res.results[0]["out"].reshape(B, D, M).transpose(0, 2, 1).copy()
